# revision 3
# baseline (speedup 1.0000x reference)
"""Trainium2 Bass kernel for nn_LocalExperts (MoE grouped FFN).

out[e] = relu(x[e] @ wi[e]) @ wo[e]   for e in 0..7

Expert-parallel over 8 NeuronCores: core e computes expert e's FFN.
Per-core work: x [8192, 512], wi [512, 2048], wo [2048, 512]
  GEMM1: hT[f, m] = wi[d, f].T @ xT[d, m]  (accumulate over 4 d-chunks)
  relu (ScalarE) -> hT in SBUF as bf16
  GEMM2: out[m, d] = hT[f, m].T @ wo[f, d] (accumulate over 16 f-chunks)

All inputs are pre-converted to bf16 on the host (max rel err of the
bf16 pipeline vs the fp32 reference is ~3e-3, measured): halves the
input DMA bytes and removes any on-chip cast pass. PSUM accumulation
is fp32; the output is written back as fp32.

x transposition: m-tiles 0-1 are transposed on the TensorE (fills the
PE during the initial weight DMA and warms the HAM clock gate);
m-tiles 2-15 arrive pre-transposed through the DMA xbar
(dma_start_transpose, DRAM -> SBUF), which takes them off the PE's
critical path entirely (~15us of PE time).

Two HWDGE rings: x loads/transposes issue on SP (nc.sync), weights
and output stores on ACT (nc.scalar), so neither flow FIFO-blocks the
other. wi is DMA'd in f-halves so GEMM1 fc=0..7 can start after half
the bytes.
"""

import numpy as np
import ml_dtypes

import concourse.mybir as mybir
from concourse import bacc
from concourse.tile import TileContext
from concourse.bass_utils import run_bass_kernel_spmd
from concourse.masks import make_identity

E, W, C, D, F = 8, 8, 1024, 512, 2048
P = 128
M_TOT = W * C            # 8192 rows per expert
M_TILE = 512             # rows per m-tile (PSUM fp32 bank = 512 cols)
N_MT = M_TOT // M_TILE   # 16
MS = M_TILE // P         # 4 m-subtiles of 128 rows
DC = D // P              # 4 d-chunks
FC = F // P              # 16 f-chunks

F32 = mybir.dt.float32
BF16 = mybir.dt.bfloat16


def _build_nc():
    nc = bacc.Bacc(None, target_bir_lowering=False)

    x = nc.dram_tensor("x", [M_TOT, D], BF16, kind="ExternalInput")
    wi = nc.dram_tensor("wi", [D, F], BF16, kind="ExternalInput")
    wo = nc.dram_tensor("wo", [F, D], BF16, kind="ExternalInput")
    out = nc.dram_tensor("out", [M_TOT, D], F32, kind="ExternalOutput")

    x_v = x.rearrange("(mt ms p) d -> mt p ms d", p=P, ms=MS)
    out_v = out.rearrange("(mt ms p) d -> mt p ms d", p=P, ms=MS)
    wi_v = wi.rearrange("(dc p) f -> p dc f", p=P)
    wo_v = wo.rearrange("(fc p) d -> p fc d", p=P)

    with TileContext(nc) as tc:
        with (
            tc.tile_pool(name="const", bufs=1) as cpool,
            tc.tile_pool(name="xin", bufs=2) as xin_pool,
            tc.tile_pool(name="xt", bufs=5) as xt_pool,
            tc.tile_pool(name="ht", bufs=2) as ht_pool,
            tc.tile_pool(name="osb", bufs=4) as o_pool,
            tc.tile_pool(name="tp_ps", bufs=2, space="PSUM") as tp_psum,
            tc.tile_pool(name="h_ps", bufs=2, space="PSUM") as h_psum,
            tc.tile_pool(name="o_ps", bufs=2, space="PSUM") as o_psum,
        ):
            ident = cpool.tile([P, P], BF16)
            make_identity(nc, ident)

            wi_sb = cpool.tile([P, DC, F], BF16)
            wo_sb = cpool.tile([P, FC, D], BF16)

            def load_x(mt):
                x_nat = xin_pool.tile([P, MS, D], BF16)
                nc.sync.dma_start(x_nat, x_v[mt])
                return x_nat

            def load_xt(mt):
                # pre-transposed load through the DMA xbar: per d-chunk,
                # x[m-tile, dc] (512x128) lands as xt[dc] (128x512).
                xt = xt_pool.tile([P, DC, M_TILE], BF16)
                for dc in range(DC):
                    nc.sync.dma_start_transpose(
                        xt[:, dc],
                        x[mt * M_TILE : (mt + 1) * M_TILE, dc * P : (dc + 1) * P],
                    )
                return xt

            # SP ring: x0, x1 (natural layout, PE-transposed), then the
            # xbar prefetches. ACT ring (parallel): wi halves, wo quarters.
            x0 = load_x(0)
            x1 = load_x(1)
            for h in range(2):
                s = slice(h * (F // 2), (h + 1) * (F // 2))
                nc.scalar.dma_start(wi_sb[:, :, s], wi_v[:, :, s])
            for q in range(4):
                s = slice(q * (FC // 4), (q + 1) * (FC // 4))
                nc.scalar.dma_start(wo_sb[:, s], wo_v[:, s])

            def transpose_x(x_nat):
                # xT [d, m] on the PE: per m-subtile, 4 transposes form ONE
                # psum accumulation group, drained by ONE DVE copy.
                xt = xt_pool.tile([P, DC, M_TILE], BF16)
                for ms in range(MS):
                    tp = tp_psum.tile([P, DC, P], BF16)
                    for dc in range(DC):
                        nc.tensor.matmul(
                            tp[:, dc],
                            x_nat[:, ms, dc * P : (dc + 1) * P],
                            ident,
                            is_transpose=True,
                            start=(dc == 0),
                            stop=(dc == DC - 1),
                            skip_group_check=True,
                        )
                    nc.vector.tensor_copy(xt[:, :, ms * P : (ms + 1) * P], tp)
                return xt

            def gemm1(xt):
                # hT[f, m]; two 4-matmul PSUM groups (adjacent banks of one
                # 2-bank tile) drained by a single ACT relu -> bf16 SBUF.
                hT = ht_pool.tile([P, FC, M_TILE], BF16)
                for fc2 in range(FC // 2):
                    hp = h_psum.tile([P, 2, M_TILE], F32)
                    for half in range(2):
                        fc = 2 * fc2 + half
                        for dc in range(DC):
                            nc.tensor.matmul(
                                hp[:, half],
                                wi_sb[:, dc, fc * P : (fc + 1) * P],
                                xt[:, dc, :],
                                start=(dc == 0),
                                stop=(dc == DC - 1),
                            )
                    nc.scalar.activation(
                        hT[:, 2 * fc2 : 2 * fc2 + 2, :],
                        hp,
                        mybir.ActivationFunctionType.Relu,
                    )
                return hT

            def gemm2(mt, hT):
                # out[m, d] per 128-row subtile; fc ascending so the last
                # relu chunk is only needed by the final two matmuls.
                for ms in range(MS):
                    op = o_psum.tile([P, D], F32)
                    for fc in range(FC):
                        nc.tensor.matmul(
                            op,
                            hT[:, fc, ms * P : (ms + 1) * P],
                            wo_sb[:, fc, :],
                            start=(fc == 0),
                            stop=(fc == FC - 1),
                        )
                    o_t = o_pool.tile([P, D], F32)
                    nc.vector.tensor_copy(o_t, op)
                    nc.scalar.dma_start(out_v[mt, :, ms, :], o_t)

            xts = {0: transpose_x(x0)}
            xts[2] = load_xt(2)
            xts[3] = load_xt(3)
            for mt in range(N_MT):
                hT = gemm1(xts.pop(mt))
                if mt == 0:
                    # PE-transpose tile 1 between G1(0) and G2(0): hides the
                    # last relu's latency and keeps the PE warm.
                    xts[1] = transpose_x(x1)
                if mt + 4 < N_MT:
                    xts[mt + 4] = load_xt(mt + 4)
                gemm2(mt, hT)

    nc.finalize()
    return nc


_CACHE = {}


def _get_nc():
    if "nc" not in _CACHE:
        _CACHE["nc"] = _build_nc()
    return _CACHE["nc"]


def _run(x, wi, wo, **spmd_kwargs):
    """x [E, 8192, 512], wi [E, 512, 2048], wo [E, 2048, 512] -> results."""
    nc = _get_nc()
    x_bf = np.asarray(x, dtype=np.float32).astype(ml_dtypes.bfloat16)
    wi_bf = np.asarray(wi, dtype=np.float32).astype(ml_dtypes.bfloat16)
    wo_bf = np.asarray(wo, dtype=np.float32).astype(ml_dtypes.bfloat16)
    in_maps = [
        {
            "x": np.ascontiguousarray(x_bf[e]),
            "wi": np.ascontiguousarray(wi_bf[e]),
            "wo": np.ascontiguousarray(wo_bf[e]),
        }
        for e in range(E)
    ]
    return nc, run_bass_kernel_spmd(nc, in_maps, core_ids=list(range(E)), **spmd_kwargs)


def kernel(dispatched_hidden_states, experts_capacity_usage=None, wi=None, wo=None):
    x = np.asarray(dispatched_hidden_states, dtype=np.float32).reshape(E, M_TOT, D)
    wi_ = np.asarray(wi, dtype=np.float32)
    wo_ = np.asarray(wo, dtype=np.float32)
    _, res = _run(x, wi_, wo_)
    out = np.stack([res.results[e]["out"] for e in range(E)])
    return out.reshape(E, W, C, D)
